# revision 23
# baseline (speedup 1.0000x reference)
# Distributed GQA attention prefill kernel for one TRN2 chip (8 NeuronCores).
#
# Problem: B=2, S=1024, D=2048, H=32 q-heads, KV=4 kv-heads, HD=64, causal,
# RoPE, f32 I/O. Sharding: core d = (batch g=d//4, kv-head kv=d%4). Each core
# computes q-proj for its 8 q heads, k/v-proj for its kv head, attention for
# 8 heads, and the full-channel o_proj for its 512 output columns, fed by one
# bf16 AllGather of oT channels per 128-row seq chunk across its group of 4.
#
# Structure (v2, per-chunk pipeline):
# - Everything is organized around 128-row seq chunks s=0..7. Chunk s needs
#   only q chunk s and k/v chunks 0..s, so scores for chunk 0 start ~12us in
#   and the 8 AllGathers (one per chunk, [512,128] bf16 payload) fire spread
#   across the whole attention window instead of bunched at the end.
# - Per chunk: q-proj (16 MMs) -> RoPE -> 2 batched transposes -> scores
#   t=0..s ([64,128,1024] MMs) -> one fused exp per t ([128,1024]) -> AV
#   accumulate into one [128,1024] PSUM (vaug carries 64 ones-rows so the
#   softmax denominators come out pre-broadcast) -> normalize -> ship ->
#   AllGather -> (lagged by 2 chunks) o_proj readback + 16 MMs + store.
# - Engine roles: PE matmuls dense in emission order; Scalar = exps + small
#   PSUM copies + ship triggers; Vector = RoPE, qT copies, mask, normalize,
#   out copies; Sync = bulk loads, readbacks, out writes; GpSimd = collective
#   chain. PSUM: scores [128,1024]x2, AV accum x1, pp(pkv/pq/tp/po) x2 = 8 banks.
import sys

import numpy as np

try:
    import concourse.bass as bass  # noqa: F401
except ImportError:
    for p in ("/opt/trn_rl_repo", "/root/.axon_site/_ro/trn_rl_repo"):
        if p not in sys.path:
            sys.path.append(p)
    import concourse.bass as bass  # noqa: F401

import concourse.bacc as bacc
import concourse.mybir as mybir
import concourse.tile as tile
from concourse import masks
from concourse.bass_utils import run_bass_kernel_spmd

S = 1024
D = 2048
H = 32
KV = 4
HD = 64
NH = 8  # q heads per core
P = 128
SC = S // P  # 8 seq chunks
DC = D // P  # 16 D chunks
N_CORES = 8
LAG = 2  # chunks between AG trigger and its o_proj emission
GROUPS = [[0, 1, 2, 3], [4, 5, 6, 7]]
AVLAG = 1  # attn@v trails the score/exp stream by this many key chunks

F32 = mybir.dt.float32
BF16 = mybir.dt.bfloat16

_NC_CACHE = {}


def _build_graph():
    nc = bacc.Bacc("TRN2", target_bir_lowering=False, debug=False, num_devices=N_CORES)

    xt_p = nc.dram_tensor("xt", [SC, P, DC * P], BF16, kind="ExternalInput")
    wq_p = nc.dram_tensor("wq", [P, DC * 512], BF16, kind="ExternalInput")
    wkv_p = nc.dram_tensor("wkv", [P, DC * 2 * HD], BF16, kind="ExternalInput")
    wo_p = nc.dram_tensor("wo", [P, DC * 512], BF16, kind="ExternalInput")
    cs_p = nc.dram_tensor("cs9", [P, SC * 288], BF16, kind="ExternalInput")
    sn_p = nc.dram_tensor("sn9", [P, SC * 288], BF16, kind="ExternalInput")
    mk_p = nc.dram_tensor("mk8", [P, SC * NH * P], BF16, kind="ExternalInput")
    out_p = nc.dram_tensor("out", [S, 512], F32, kind="ExternalOutput")

    with tile.TileContext(nc) as tc:
        with (
            tc.tile_pool(name="const", bufs=1) as constp,
            tc.tile_pool(name="big", bufs=1) as bigp,
            tc.tile_pool(name="work", bufs=1) as workp,
            tc.tile_pool(name="rt", bufs=4) as rtp,
            tc.tile_pool(name="attn", bufs=3) as attnp,
            tc.tile_pool(name="opart", bufs=2) as opartp,
            tc.tile_pool(name="tiny", bufs=1) as tinyp,
            tc.tile_pool(name="psum", bufs=1, space="PSUM") as psump,
            tc.tile_pool(name="dram", bufs=1, space="DRAM") as dramp,
        ):
            ident = constp.tile([P, P], BF16, tag="ident")
            masks.make_identity(nc, ident[:])

            xT_all = bigp.tile([P, SC * DC * P], BF16, tag="xT_all")
            wkv_all = bigp.tile([P, DC * 2 * HD], BF16, tag="wkv_all")
            wq_all = bigp.tile([P, DC * 512], BF16, tag="wq_all")
            wo_all = bigp.tile([P, DC * 512], BF16, tag="wo_all")
            cst = constp.tile([P, SC * 288], BF16, tag="cst")
            snt = constp.tile([P, SC * 288], BF16, tag="snt")
            mkt = constp.tile([P, SC * NH * P], BF16, tag="mkt")

            # Load order tracks first use. HWDGE triggers pace with their
            # ring's transfers, so each engine only carries loads it can
            # afford to wait on: Scalar the tiny early set (before its first
            # copy at ~10us), Sync the mid set (its ships start ~22us),
            # GpSimd (slow SWDGE, otherwise idle) the late bulk (x chunks
            # 4-7, late mask blocks, wo).
            nc.scalar.dma_start(out=wkv_all[:, 0:1024], in_=wkv_p[:, 0:1024])
            nc.sync.dma_start(out=xT_all[:, 0:D], in_=xt_p[0, :, :])
            nc.scalar.dma_start(out=wq_all[:, 0:2048], in_=wq_p[:, 0:2048])
            nc.sync.dma_start(out=wq_all[:, 4096:6144], in_=wq_p[:, 4096:6144])
            nc.scalar.dma_start(out=wq_all[:, 2048:4096], in_=wq_p[:, 2048:4096])
            nc.sync.dma_start(out=wq_all[:, 6144:8192], in_=wq_p[:, 6144:8192])
            nc.scalar.dma_start(out=wkv_all[:, 1024:2048], in_=wkv_p[:, 1024:2048])
            nc.sync.dma_start(out=snt[:], in_=sn_p[:, :])
            nc.scalar.dma_start(out=cst[:], in_=cs_p[:, :])
            nc.sync.dma_start(out=mkt[:, 0:2048], in_=mk_p[:, 0:2048])
            for s in range(1, 4):
                nc.sync.dma_start(out=xT_all[:, s * D : (s + 1) * D], in_=xt_p[s, :, :])

            def emit_deferred_loads(s):
                # bulk loads paced through the loop on Sync so they do not
                # compete with the startup critical path for HBM bandwidth
                if s == 0:
                    nc.sync.dma_start(out=mkt[:, 2048:4096], in_=mk_p[:, 2048:4096])
                    nc.sync.dma_start(
                        out=xT_all[:, 4 * D : 5 * D], in_=xt_p[4, :, :]
                    )
                elif s == 1:
                    nc.sync.dma_start(out=mkt[:, 4096:8192], in_=mk_p[:, 4096:8192])
                    nc.sync.dma_start(
                        out=xT_all[:, 5 * D : 6 * D], in_=xt_p[5, :, :]
                    )
                elif s in (2, 3):
                    nc.sync.dma_start(
                        out=xT_all[:, (s + 4) * D : (s + 5) * D], in_=xt_p[s + 4, :, :]
                    )
                    g = s - 2
                    nc.sync.dma_start(
                        out=wo_all[:, g * 2048 : (g + 1) * 2048],
                        in_=wo_p[:, g * 2048 : (g + 1) * 2048],
                    )
                elif s == 4:
                    for g in (2, 3):
                        nc.sync.dma_start(
                            out=wo_all[:, g * 2048 : (g + 1) * 2048],
                            in_=wo_p[:, g * 2048 : (g + 1) * 2048],
                        )

            # ---- persistent per-chunk state ----
            vaug = [workp.tile([P, 2 * HD], BF16, tag=f"va{s}", name=f"va{s}") for s in range(SC)]
            # kT/qTall carry K=128 with rows 64:128 zeroed: mixing K=64 and
            # K=128 matmuls makes the PE row-group power state (HAM)
            # oscillate and halves throughput; zero-padded K=128 costs the
            # same cycles and keeps the array warm.
            kT = workp.tile([P, S], BF16, tag="kT")
            qTall = workp.tile([P, NH * S], BF16, tag="qTall")
            # qkrot[s]: roped q (cols 0:512, 8 heads) + roped k (cols 512:576)
            qkrot = [workp.tile([P, 576], BF16, tag=f"qk{s}", name=f"qk{s}") for s in range(SC)]
            # scores rhs view: [64, sc, h, b] -> chunk s gives [64, 8, 128]
            qview = qTall[:].rearrange("p (h sc b) -> p sc h b", h=NH, sc=SC, b=P)

            for s in range(SC):
                nc.gpsimd.memset(vaug[s][:, HD : 2 * HD], 1.0)
            nc.gpsimd.memset(kT[64:P, :], 0.0)
            nc.gpsimd.memset(qTall[64:P, :], 0.0)
            agin = [dramp.tile([NH * HD, 256], BF16, name=f"agin{p}") for p in range(SC // 2)]
            agout = [dramp.tile([D, 256], BF16, name=f"agout{p}") for p in range(SC // 2)]

            def emit_qkv(s):
                """Fused q+k+v projection for chunk s into one [128,640] PSUM
                (q 8 heads in 0:512, k in 512:576, v in 576:640), one 9-head
                RoPE pass over cols 0:576, transposes into qTall/kT, v copy
                into vaug."""
                pqkv = psump.tile([P, 640], F32, tag="pqkv", bufs=1, name=f"pqkv{s}")
                for d in range(DC):
                    xt_d = xT_all[:, s * D + d * P : s * D + (d + 1) * P]
                    nc.tensor.matmul(
                        pqkv[:, 0:512],
                        xt_d,
                        wq_all[:, d * 512 : (d + 1) * 512],
                        start=(d == 0),
                        stop=(d == DC - 1),
                        skip_group_check=True,
                    )
                    nc.tensor.matmul(
                        pqkv[:, 512:640],
                        xt_d,
                        wkv_all[:, d * 2 * HD : (d + 1) * 2 * HD],
                        start=(d == 0),
                        stop=(d == DC - 1),
                        skip_group_check=True,
                    )
                # 9-head rope (8 q heads + k) in 6 vector ops
                nh = 9
                pv = pqkv[:, 0 : nh * 64].rearrange("p (h t c) -> p h t c", h=nh, t=2)
                dv = qkrot[s][:].rearrange("p (h t c) -> p h t c", h=nh, t=2)
                cs = cst[:, s * 288 : s * 288 + nh * 32].rearrange(
                    "p (h c) -> p h c", h=nh
                )
                sn = snt[:, s * 288 : s * 288 + nh * 32].rearrange(
                    "p (h c) -> p h c", h=nh
                )
                lo, hi = pv[:, :, 0, :], pv[:, :, 1, :]
                t1 = rtp.tile([P, nh * 32], F32, tag="rt1")
                t2 = rtp.tile([P, nh * 32], F32, tag="rt2")
                t1v = t1[:].rearrange("p (h c) -> p h c", h=nh)
                t2v = t2[:].rearrange("p (h c) -> p h c", h=nh)
                nc.vector.tensor_mul(t1v, lo, cs)
                nc.vector.tensor_mul(t2v, hi, sn)
                nc.vector.tensor_sub(dv[:, :, 0, :], t1v, t2v)
                nc.vector.tensor_mul(t1v, hi, cs)
                nc.vector.tensor_mul(t2v, lo, sn)
                nc.vector.tensor_add(dv[:, :, 1, :], t1v, t2v)
                nc.scalar.copy(vaug[s][:, 0:HD], pqkv[:, 576:640])
                # k transpose
                tpk = psump.tile([64, 512], BF16, tag="sc", bufs=2, name=f"tpk{s}")
                nc.tensor.transpose(tpk[:, 0:P], qkrot[s][:, 512:576], ident[:])
                nc.scalar.copy(kT[0:64, s * P : (s + 1) * P], tpk[:, 0:P])
                # q transposes: 4 heads per PSUM tile, strided copy out
                for half in range(2):
                    tpq = psump.tile([64, 512], BF16, tag="sc", bufs=2, name=f"tpq{s}_{half}")
                    for hh in range(4):
                        h = 4 * half + hh
                        nc.tensor.transpose(
                            tpq[:, hh * P : (hh + 1) * P],
                            qkrot[s][:, h * HD : (h + 1) * HD],
                            ident[:],
                        )
                    dst = qview[0:64, s, 4 * half : 4 * half + 4, :]
                    src = tpq[:].rearrange("p (h b) -> p h b", h=4)
                    nc.vector.tensor_copy(dst, src)

            oa_live = {}

            def emit_av(s, ats, t0, t1):
                """attn@v for chunk s, key chunks t0..t1-1, into the chunk's
                [128,1024] accumulator (rows 0:64 = oT, 64:128 = denom —
                vaug carries 64 ones-rows so denominators land
                pre-broadcast)."""
                if s not in oa_live:
                    oa_live[s] = psump.tile(
                        [P, 1024], F32, tag="av", bufs=1, name=f"oa{s}"
                    )
                oa = oa_live[s]
                for t in range(t0, t1):
                    for half in range(2):
                        nc.tensor.matmul(
                            oa[:, half * 512 : (half + 1) * 512],
                            vaug[t][:],
                            ats[t][:, half * 512 : (half + 1) * 512],
                            start=(t == 0),
                            stop=(t == s),
                            skip_group_check=True,
                        )

            def emit_scores_av(s, after_first=None):
                """score matmuls + fused exps for chunk s vs key chunks
                t=0..s, with attn@v for t-1 interleaved right behind so PE
                stays busy while ACT streams exps; after_first() (the
                previous chunk's normalize/ship) is emitted after the t=0
                exp so its Scalar copies don't gate this chunk's exp
                stream. Leaves AV for t=s-1 and the masked diag to the
                caller."""
                ats = []
                for t in range(s + 1):
                    sc2 = psump.tile([P, 1024], F32, tag="sc", bufs=2)
                    for half in range(2):
                        nc.tensor.matmul(
                            sc2[:, half * 512 : (half + 1) * 512],
                            kT[:, t * P : (t + 1) * P],
                            qview[:, s, 4 * half : 4 * half + 4, :],
                            start=True,
                            stop=True,
                        )
                    at2 = attnp.tile([P, 1024], BF16, tag="at", bufs=6)
                    nc.scalar.activation(
                        at2[:], sc2[:], mybir.ActivationFunctionType.Exp, scale=0.125
                    )
                    ats.append(at2)
                    if t == 0 and after_first is not None:
                        after_first()
                    if t >= AVLAG:
                        emit_av(s, ats, t - AVLAG, t - AVLAG + 1)
                return ats

            def emit_norm_ship(s):
                oa = oa_live.pop(s)
                # ACT stages numerator and denominators at base partition 0
                # (reciprocal_approx_fast misreads partition-offset inputs);
                # reciprocal + 2 strided batched normalize muls (even/odd
                # heads) on Vector.
                onum = tinyp.tile([64, 1024], F32, tag="onum", bufs=2)
                nc.scalar.copy(onum[:], oa[0:HD, :])
                den0 = tinyp.tile([64, 1024], F32, tag="den0", bufs=2)
                nc.scalar.copy(den0[:], oa[HD : 2 * HD, :])
                rc2 = tinyp.tile([64, 1024], F32, tag="rc2", bufs=2)
                nc.vector.reciprocal_approx_fast(rc2[:], den0[:])
                nv = onum[:].rearrange("p (c two n) -> p two c n", two=2, n=P)
                rv = rc2[:].rearrange("p (c two n) -> p two c n", two=2, n=P)
                ote = opartp.tile([64, 512], BF16, tag="ote", bufs=2)
                oto = opartp.tile([64, 512], BF16, tag="oto", bufs=2)
                nc.vector.tensor_mul(
                    ote[:].rearrange("p (c n) -> p c n", n=P), nv[:, 0], rv[:, 0]
                )
                nc.vector.tensor_mul(
                    oto[:].rearrange("p (c n) -> p c n", n=P), nv[:, 1], rv[:, 1]
                )
                pr, k = s // 2, s % 2
                dstv = agin[pr][:, k * P : (k + 1) * P].rearrange(
                    "(c two q) n -> two q c n", two=2, q=64
                )
                nc.sync.dma_start(
                    out=dstv[0], in_=ote[:].rearrange("p (c n) -> p c n", n=P)
                )
                nc.sync.dma_start(
                    out=dstv[1], in_=oto[:].rearrange("p (c n) -> p c n", n=P)
                )
                if k == 1:
                    nc.gpsimd.collective_compute(
                        "AllGather",
                        mybir.AluOpType.bypass,
                        replica_groups=GROUPS,
                        ins=[agin[pr].opt()],
                        outs=[agout[pr].opt()],
                    )

            def emit_oproj(s):
                pr, k = s // 2, s % 2
                ag_sb = opartp.tile([P, DC * P], BF16, tag="agsb", bufs=2)
                eng = nc.sync if s % 2 == 0 else nc.scalar
                eng.dma_start(
                    out=ag_sb[:].rearrange("p (c n) -> p c n", c=DC),
                    in_=agout[pr][:, k * P : (k + 1) * P].rearrange(
                        "(c p) n -> p c n", p=P
                    ),
                )
                po = psump.tile([P, 512], F32, tag="sc", bufs=2, name=f"po{s}")
                for c in range(DC):
                    nc.tensor.matmul(
                        po[:],
                        ag_sb[:, c * P : (c + 1) * P],
                        wo_all[:, c * 512 : (c + 1) * 512],
                        start=(c == 0),
                        stop=(c == DC - 1),
                    )
                osb = opartp.tile([P, 512], F32, tag="osb", bufs=2)
                nc.vector.tensor_copy(osb[:], po[:])
                eng.dma_start(out=out_p[s * P : (s + 1) * P, :], in_=osb[:])

            # ---- main per-chunk pipeline ----
            # Per chunk: scores+exps stream with AV interleaved one step
            # behind; the NEXT chunk's fused qkv projection + rope +
            # transposes fill PE during the exp stream; then the masked diag
            # AV closes the accumulator and normalize/ship run spread over
            # Scalar/Vector/GpSimd. One AllGather per pair of chunks; all
            # o_proj work happens at the end when every AG has completed.
            emit_qkv(0)
            for s in range(SC):
                ats = emit_scores_av(s)
                if s + 1 < SC:
                    emit_qkv(s + 1)
                nc.vector.tensor_mul(
                    ats[s][:], ats[s][:], mkt[:, s * 1024 : (s + 1) * 1024]
                )
                emit_av(s, ats, s, s + 1)
                emit_norm_ship(s)
                emit_deferred_loads(s)
            for s in range(SC):
                emit_oproj(s)

    nc.compile()
    return nc


def _get_nc():
    if "nc" not in _NC_CACHE:
        _NC_CACHE["nc"] = _build_graph()
    return _NC_CACHE["nc"]


def _shard_inputs(x, wq, wk, wv, wo, cos, sin, mask, pos):
    import ml_dtypes

    bf16 = ml_dtypes.bfloat16
    x = np.asarray(x, dtype=np.float32).astype(bf16)
    wq = np.asarray(wq, dtype=np.float32).astype(bf16)
    wk = np.asarray(wk, dtype=np.float32).astype(bf16)
    wv = np.asarray(wv, dtype=np.float32).astype(bf16)
    wo = np.asarray(wo, dtype=np.float32).astype(bf16)
    cos = np.asarray(cos, dtype=np.float32)
    sin = np.asarray(sin, dtype=np.float32)
    mask = np.asarray(mask, dtype=np.float32)
    p = int(pos)

    def pblock(a, nchunks):
        # [(chunks*128), n] -> [128, chunks, n] -> [128, chunks*n]
        n = a.shape[1]
        return np.ascontiguousarray(
            a.reshape(nchunks, P, n).transpose(1, 0, 2).reshape(P, nchunks * n)
        )

    cs = cos[p : p + S]  # [S, 32]
    sn = sin[p : p + S]
    cs9 = pblock(np.tile(cs, (1, NH + 1)), SC).astype(bf16)  # [128, 8*288]
    sn9 = pblock(np.tile(sn, (1, NH + 1)), SC).astype(bf16)
    # transposed diagonal 128x128 blocks of the mask as 0/1, tiled x8 heads
    mk8 = np.concatenate(
        [
            np.tile(
                (mask[s * P : (s + 1) * P, s * P : (s + 1) * P].T >= -0.5).astype(
                    bf16
                ),
                (1, NH),
            )
            for s in range(SC)
        ],
        axis=1,
    )
    mk8 = np.ascontiguousarray(mk8)  # [128, 8*1024]

    in_maps = []
    for d in range(N_CORES):
        g, kv = d // 4, d % 4
        in_maps.append(
            {
                "xt": np.ascontiguousarray(
                    x[g].T.reshape(DC, P, SC, P).transpose(2, 1, 0, 3).reshape(SC, P, D)
                ),
                "wq": pblock(wq[:, kv * 512 : (kv + 1) * 512], DC),
                "wkv": pblock(
                    np.concatenate(
                        [
                            wk[:, kv * HD : (kv + 1) * HD],
                            wv[:, kv * HD : (kv + 1) * HD],
                        ],
                        axis=1,
                    ),
                    DC,
                ),
                "wo": pblock(wo[:, kv * 512 : (kv + 1) * 512], DC),
                "cs9": cs9,
                "sn9": sn9,
                "mk8": mk8,
            }
        )
    return in_maps


def _run(inputs, trace=False, trace_kwargs=None):
    nc = _get_nc()
    in_maps = _shard_inputs(**inputs)
    res = run_bass_kernel_spmd(
        nc,
        in_maps,
        core_ids=list(range(N_CORES)),
        trace=trace,
        **(trace_kwargs or {}),
    )
    B = 2
    out = np.empty((B, S, D), dtype=np.float32)
    for d in range(N_CORES):
        g, kv = d // 4, d % 4
        out[g, :, kv * 512 : (kv + 1) * 512] = res.results[d]["out"]
    return out, res


def kernel(**inputs) -> np.ndarray:
    out, _ = _run(inputs, trace=False)
    return out


# revision 24
# speedup vs baseline: 1.0506x; 1.0506x over previous
# Distributed GQA attention prefill kernel for one TRN2 chip (8 NeuronCores).
#
# Problem: B=2, S=1024, D=2048, H=32 q-heads, KV=4 kv-heads, HD=64, causal,
# RoPE, f32 I/O. Sharding: core d = (batch g=d//4, kv-head kv=d%4). Each core
# computes q-proj for its 8 q heads, k/v-proj for its kv head, attention for
# 8 heads, and the full-channel o_proj for its 512 output columns, fed by one
# bf16 AllGather of oT channels per 128-row seq chunk across its group of 4.
#
# Structure (v2, per-chunk pipeline):
# - Everything is organized around 128-row seq chunks s=0..7. Chunk s needs
#   only q chunk s and k/v chunks 0..s, so scores for chunk 0 start ~12us in
#   and the 8 AllGathers (one per chunk, [512,128] bf16 payload) fire spread
#   across the whole attention window instead of bunched at the end.
# - Per chunk: q-proj (16 MMs) -> RoPE -> 2 batched transposes -> scores
#   t=0..s ([64,128,1024] MMs) -> one fused exp per t ([128,1024]) -> AV
#   accumulate into one [128,1024] PSUM (vaug carries 64 ones-rows so the
#   softmax denominators come out pre-broadcast) -> normalize -> ship ->
#   AllGather -> (lagged by 2 chunks) o_proj readback + 16 MMs + store.
# - Engine roles: PE matmuls dense in emission order; Scalar = exps + small
#   PSUM copies + ship triggers; Vector = RoPE, qT copies, mask, normalize,
#   out copies; Sync = bulk loads, readbacks, out writes; GpSimd = collective
#   chain. PSUM: scores [128,1024]x2, AV accum x1, pp(pkv/pq/tp/po) x2 = 8 banks.
import sys

import numpy as np

try:
    import concourse.bass as bass  # noqa: F401
except ImportError:
    for p in ("/opt/trn_rl_repo", "/root/.axon_site/_ro/trn_rl_repo"):
        if p not in sys.path:
            sys.path.append(p)
    import concourse.bass as bass  # noqa: F401

import concourse.bacc as bacc
import concourse.mybir as mybir
import concourse.tile as tile
from concourse import masks
from concourse.bass_utils import run_bass_kernel_spmd

S = 1024
D = 2048
H = 32
KV = 4
HD = 64
NH = 8  # q heads per core
P = 128
SC = S // P  # 8 seq chunks
DC = D // P  # 16 D chunks
N_CORES = 8
LAG = 2  # chunks between AG trigger and its o_proj emission
GROUPS = [[0, 1, 2, 3], [4, 5, 6, 7]]
AVLAG = 1  # attn@v trails the score/exp stream by this many key chunks

F32 = mybir.dt.float32
BF16 = mybir.dt.bfloat16

_NC_CACHE = {}


def _build_graph():
    nc = bacc.Bacc("TRN2", target_bir_lowering=False, debug=False, num_devices=N_CORES)

    xt_p = nc.dram_tensor("xt", [SC, P, DC * P], BF16, kind="ExternalInput")
    wq_p = nc.dram_tensor("wq", [P, DC * 512], BF16, kind="ExternalInput")
    wkv_p = nc.dram_tensor("wkv", [P, DC * 2 * HD], BF16, kind="ExternalInput")
    wo_p = nc.dram_tensor("wo", [P, DC * 512], BF16, kind="ExternalInput")
    cs_p = nc.dram_tensor("cs9", [P, SC * 288], BF16, kind="ExternalInput")
    sn_p = nc.dram_tensor("sn9", [P, SC * 288], BF16, kind="ExternalInput")
    mk_p = nc.dram_tensor("mk8", [P, SC * NH * P], BF16, kind="ExternalInput")
    out_p = nc.dram_tensor("out", [S, 512], F32, kind="ExternalOutput")

    with tile.TileContext(nc) as tc:
        with (
            tc.tile_pool(name="const", bufs=1) as constp,
            tc.tile_pool(name="big", bufs=1) as bigp,
            tc.tile_pool(name="work", bufs=1) as workp,
            tc.tile_pool(name="rt", bufs=4) as rtp,
            tc.tile_pool(name="attn", bufs=3) as attnp,
            tc.tile_pool(name="opart", bufs=2) as opartp,
            tc.tile_pool(name="tiny", bufs=1) as tinyp,
            tc.tile_pool(name="psum", bufs=1, space="PSUM") as psump,
            tc.tile_pool(name="dram", bufs=1, space="DRAM") as dramp,
        ):
            ident = constp.tile([P, P], BF16, tag="ident")
            masks.make_identity(nc, ident[:])

            xT_all = bigp.tile([P, SC * DC * P], BF16, tag="xT_all")
            wkv_all = bigp.tile([P, DC * 2 * HD], BF16, tag="wkv_all")
            wq_all = bigp.tile([P, DC * 512], BF16, tag="wq_all")
            wo_all = bigp.tile([P, DC * 512], BF16, tag="wo_all")
            cst = constp.tile([P, SC * 288], BF16, tag="cst")
            snt = constp.tile([P, SC * 288], BF16, tag="snt")
            mkt = constp.tile([P, SC * NH * P], BF16, tag="mkt")

            # Load order tracks first use. HWDGE triggers pace with their
            # ring's transfers, so each engine only carries loads it can
            # afford to wait on: Scalar the tiny early set (before its first
            # copy at ~10us), Sync the mid set (its ships start ~22us),
            # GpSimd (slow SWDGE, otherwise idle) the late bulk (x chunks
            # 4-7, late mask blocks, wo).
            nc.scalar.dma_start(out=wkv_all[:, 0:1024], in_=wkv_p[:, 0:1024])
            nc.sync.dma_start(out=xT_all[:, 0:D], in_=xt_p[0, :, :])
            nc.scalar.dma_start(out=wq_all[:, 0:2048], in_=wq_p[:, 0:2048])
            nc.sync.dma_start(out=wq_all[:, 4096:6144], in_=wq_p[:, 4096:6144])
            nc.scalar.dma_start(out=wq_all[:, 2048:4096], in_=wq_p[:, 2048:4096])
            nc.sync.dma_start(out=wq_all[:, 6144:8192], in_=wq_p[:, 6144:8192])
            nc.scalar.dma_start(out=wkv_all[:, 1024:2048], in_=wkv_p[:, 1024:2048])
            nc.sync.dma_start(out=snt[:], in_=sn_p[:, :])
            nc.scalar.dma_start(out=cst[:], in_=cs_p[:, :])
            nc.sync.dma_start(out=mkt[:, 0:2048], in_=mk_p[:, 0:2048])
            for s in range(1, 4):
                nc.sync.dma_start(out=xT_all[:, s * D : (s + 1) * D], in_=xt_p[s, :, :])

            def emit_deferred_loads(s):
                # bulk loads paced through the loop on Sync so they do not
                # compete with the startup critical path for HBM bandwidth
                if s == 0:
                    nc.sync.dma_start(out=mkt[:, 2048:4096], in_=mk_p[:, 2048:4096])
                    nc.sync.dma_start(
                        out=xT_all[:, 4 * D : 5 * D], in_=xt_p[4, :, :]
                    )
                elif s == 1:
                    nc.sync.dma_start(out=mkt[:, 4096:8192], in_=mk_p[:, 4096:8192])
                    nc.sync.dma_start(
                        out=xT_all[:, 5 * D : 6 * D], in_=xt_p[5, :, :]
                    )
                elif s in (2, 3):
                    nc.sync.dma_start(
                        out=xT_all[:, (s + 4) * D : (s + 5) * D], in_=xt_p[s + 4, :, :]
                    )
                    g = s - 2
                    nc.sync.dma_start(
                        out=wo_all[:, g * 2048 : (g + 1) * 2048],
                        in_=wo_p[:, g * 2048 : (g + 1) * 2048],
                    )
                elif s == 4:
                    for g in (2, 3):
                        nc.sync.dma_start(
                            out=wo_all[:, g * 2048 : (g + 1) * 2048],
                            in_=wo_p[:, g * 2048 : (g + 1) * 2048],
                        )

            # ---- persistent per-chunk state ----
            vaug = [workp.tile([P, 2 * HD], BF16, tag=f"va{s}", name=f"va{s}") for s in range(SC)]
            # kT/qTall carry K=128 with rows 64:128 zeroed: mixing K=64 and
            # K=128 matmuls makes the PE row-group power state (HAM)
            # oscillate and halves throughput; zero-padded K=128 costs the
            # same cycles and keeps the array warm.
            kT = workp.tile([P, S], BF16, tag="kT")
            qTall = workp.tile([P, NH * S], BF16, tag="qTall")
            # qkrot[s]: roped q (cols 0:512, 8 heads) + roped k (cols 512:576)
            qkrot = [workp.tile([P, 576], BF16, tag=f"qk{s}", name=f"qk{s}") for s in range(SC)]
            # scores rhs view: [64, sc, h, b] -> chunk s gives [64, 8, 128]
            qview = qTall[:].rearrange("p (h sc b) -> p sc h b", h=NH, sc=SC, b=P)

            for s in range(SC):
                nc.gpsimd.memset(vaug[s][:, HD : 2 * HD], 1.0)
            nc.gpsimd.memset(kT[64:P, :], 0.0)
            nc.gpsimd.memset(qTall[64:P, :], 0.0)
            agin = [dramp.tile([NH * HD, 256], BF16, name=f"agin{p}") for p in range(SC // 2)]
            agout = [dramp.tile([D, 256], BF16, name=f"agout{p}") for p in range(SC // 2)]

            def emit_qkv(s):
                """Fused q+k+v projection for chunk s into one [128,640] PSUM
                (q 8 heads in 0:512, k in 512:576, v in 576:640), one 9-head
                RoPE pass over cols 0:576, transposes into qTall/kT, v copy
                into vaug."""
                pqkv = psump.tile([P, 640], F32, tag="pqkv", bufs=1, name=f"pqkv{s}")
                for d in range(DC):
                    xt_d = xT_all[:, s * D + d * P : s * D + (d + 1) * P]
                    nc.tensor.matmul(
                        pqkv[:, 0:512],
                        xt_d,
                        wq_all[:, d * 512 : (d + 1) * 512],
                        start=(d == 0),
                        stop=(d == DC - 1),
                        skip_group_check=True,
                    )
                    nc.tensor.matmul(
                        pqkv[:, 512:640],
                        xt_d,
                        wkv_all[:, d * 2 * HD : (d + 1) * 2 * HD],
                        start=(d == 0),
                        stop=(d == DC - 1),
                        skip_group_check=True,
                    )
                # 9-head rope (8 q heads + k) in 6 vector ops
                nh = 9
                pv = pqkv[:, 0 : nh * 64].rearrange("p (h t c) -> p h t c", h=nh, t=2)
                dv = qkrot[s][:].rearrange("p (h t c) -> p h t c", h=nh, t=2)
                cs = cst[:, s * 288 : s * 288 + nh * 32].rearrange(
                    "p (h c) -> p h c", h=nh
                )
                sn = snt[:, s * 288 : s * 288 + nh * 32].rearrange(
                    "p (h c) -> p h c", h=nh
                )
                lo, hi = pv[:, :, 0, :], pv[:, :, 1, :]
                t1 = rtp.tile([P, nh * 32], F32, tag="rt1")
                t2 = rtp.tile([P, nh * 32], F32, tag="rt2")
                t1v = t1[:].rearrange("p (h c) -> p h c", h=nh)
                t2v = t2[:].rearrange("p (h c) -> p h c", h=nh)
                nc.vector.tensor_mul(t1v, lo, cs)
                nc.vector.tensor_mul(t2v, hi, sn)
                nc.vector.tensor_sub(dv[:, :, 0, :], t1v, t2v)
                nc.vector.tensor_mul(t1v, hi, cs)
                nc.vector.tensor_mul(t2v, lo, sn)
                nc.vector.tensor_add(dv[:, :, 1, :], t1v, t2v)
                nc.scalar.copy(vaug[s][:, 0:HD], pqkv[:, 576:640])
                # k transpose
                tpk = psump.tile([64, 512], BF16, tag="sc", bufs=2, name=f"tpk{s}")
                nc.tensor.transpose(tpk[:, 0:P], qkrot[s][:, 512:576], ident[:])
                nc.scalar.copy(kT[0:64, s * P : (s + 1) * P], tpk[:, 0:P])
                # q transposes: 4 heads per PSUM tile, strided copy out
                for half in range(2):
                    tpq = psump.tile([64, 512], BF16, tag="sc", bufs=2, name=f"tpq{s}_{half}")
                    for hh in range(4):
                        h = 4 * half + hh
                        nc.tensor.transpose(
                            tpq[:, hh * P : (hh + 1) * P],
                            qkrot[s][:, h * HD : (h + 1) * HD],
                            ident[:],
                        )
                    dst = qview[0:64, s, 4 * half : 4 * half + 4, :]
                    src = tpq[:].rearrange("p (h b) -> p h b", h=4)
                    nc.vector.tensor_copy(dst, src)

            oa_live = {}

            def emit_av(s, ats, t0, t1):
                """attn@v for chunk s, key chunks t0..t1-1, into the chunk's
                [128,1024] accumulator (rows 0:64 = oT, 64:128 = denom —
                vaug carries 64 ones-rows so denominators land
                pre-broadcast)."""
                if s not in oa_live:
                    oa_live[s] = psump.tile(
                        [P, 1024], F32, tag="av", bufs=1, name=f"oa{s}"
                    )
                oa = oa_live[s]
                for t in range(t0, t1):
                    for half in range(2):
                        nc.tensor.matmul(
                            oa[:, half * 512 : (half + 1) * 512],
                            vaug[t][:],
                            ats[t][:, half * 512 : (half + 1) * 512],
                            start=(t == 0),
                            stop=(t == s),
                            skip_group_check=True,
                        )

            def emit_scores_av(s):
                """score matmuls + fused exps for chunk s vs key chunks
                t=0..s, with attn@v for t-1 interleaved right behind so PE
                stays busy while ACT streams exps. Leaves AV for t=s-1 and
                the masked diag to the caller."""
                ats = []
                for t in range(s + 1):
                    sc2 = psump.tile([P, 1024], F32, tag="sc", bufs=2)
                    for half in range(2):
                        nc.tensor.matmul(
                            sc2[:, half * 512 : (half + 1) * 512],
                            kT[:, t * P : (t + 1) * P],
                            qview[:, s, 4 * half : 4 * half + 4, :],
                            start=True,
                            stop=True,
                        )
                    at2 = attnp.tile([P, 1024], BF16, tag="at", bufs=6)
                    nc.scalar.activation(
                        at2[:], sc2[:], mybir.ActivationFunctionType.Exp, scale=0.125
                    )
                    ats.append(at2)
                    if t >= AVLAG:
                        emit_av(s, ats, t - AVLAG, t - AVLAG + 1)
                return ats

            def emit_norm_ship(s):
                oa = oa_live.pop(s)
                # ACT stages numerator and denominators at base partition 0
                # (reciprocal_approx_fast misreads partition-offset inputs);
                # reciprocal + 2 strided batched normalize muls (even/odd
                # heads) on Vector.
                onum = tinyp.tile([64, 1024], F32, tag="onum", bufs=2)
                nc.scalar.copy(onum[:], oa[0:HD, :])
                den0 = tinyp.tile([64, 1024], F32, tag="den0", bufs=2)
                nc.scalar.copy(den0[:], oa[HD : 2 * HD, :])
                rc2 = tinyp.tile([64, 1024], F32, tag="rc2", bufs=2)
                nc.vector.reciprocal_approx_fast(rc2[:], den0[:])
                nv = onum[:].rearrange("p (c two n) -> p two c n", two=2, n=P)
                rv = rc2[:].rearrange("p (c two n) -> p two c n", two=2, n=P)
                ote = opartp.tile([64, 512], BF16, tag="ote", bufs=2)
                oto = opartp.tile([64, 512], BF16, tag="oto", bufs=2)
                nc.vector.tensor_mul(
                    ote[:].rearrange("p (c n) -> p c n", n=P), nv[:, 0], rv[:, 0]
                )
                nc.vector.tensor_mul(
                    oto[:].rearrange("p (c n) -> p c n", n=P), nv[:, 1], rv[:, 1]
                )
                pr, k = s // 2, s % 2
                dstv = agin[pr][:, k * P : (k + 1) * P].rearrange(
                    "(c two q) n -> two q c n", two=2, q=64
                )
                nc.sync.dma_start(
                    out=dstv[0], in_=ote[:].rearrange("p (c n) -> p c n", n=P)
                )
                nc.sync.dma_start(
                    out=dstv[1], in_=oto[:].rearrange("p (c n) -> p c n", n=P)
                )
                if k == 1:
                    nc.gpsimd.collective_compute(
                        "AllGather",
                        mybir.AluOpType.bypass,
                        replica_groups=GROUPS,
                        ins=[agin[pr].opt()],
                        outs=[agout[pr].opt()],
                    )

            def emit_oproj(s):
                pr, k = s // 2, s % 2
                ag_sb = opartp.tile([P, DC * P], BF16, tag="agsb", bufs=2)
                eng = nc.sync if s % 2 == 0 else nc.scalar
                eng.dma_start(
                    out=ag_sb[:].rearrange("p (c n) -> p c n", c=DC),
                    in_=agout[pr][:, k * P : (k + 1) * P].rearrange(
                        "(c p) n -> p c n", p=P
                    ),
                )
                po = psump.tile([P, 512], F32, tag="sc", bufs=2, name=f"po{s}")
                for c in range(DC):
                    nc.tensor.matmul(
                        po[:],
                        ag_sb[:, c * P : (c + 1) * P],
                        wo_all[:, c * 512 : (c + 1) * 512],
                        start=(c == 0),
                        stop=(c == DC - 1),
                    )
                osb = opartp.tile([P, 512], F32, tag="osb", bufs=2)
                nc.vector.tensor_copy(osb[:], po[:])
                eng.dma_start(out=out_p[s * P : (s + 1) * P, :], in_=osb[:])

            # ---- main per-chunk pipeline ----
            # Per chunk: scores+exps stream with AV interleaved one step
            # behind; the NEXT chunk's fused qkv projection + rope +
            # transposes fill PE during the exp stream; then the masked diag
            # AV closes the accumulator and normalize/ship run spread over
            # Scalar/Vector/GpSimd. One AllGather per pair of chunks; all
            # o_proj work happens at the end when every AG has completed.
            emit_qkv(0)
            for s in range(SC):
                ats = emit_scores_av(s)
                if s + 1 < SC:
                    emit_qkv(s + 1)
                nc.vector.tensor_mul(
                    ats[s][:], ats[s][:], mkt[:, s * 1024 : (s + 1) * 1024]
                )
                emit_av(s, ats, s, s + 1)
                emit_norm_ship(s)
                emit_deferred_loads(s)
            for s in range(SC):
                emit_oproj(s)

    nc.compile()
    return nc


def _get_nc():
    if "nc" not in _NC_CACHE:
        _NC_CACHE["nc"] = _build_graph()
    return _NC_CACHE["nc"]


def _shard_inputs(x, wq, wk, wv, wo, cos, sin, mask, pos):
    import ml_dtypes

    bf16 = ml_dtypes.bfloat16
    x = np.asarray(x, dtype=np.float32).astype(bf16)
    wq = np.asarray(wq, dtype=np.float32).astype(bf16)
    wk = np.asarray(wk, dtype=np.float32).astype(bf16)
    wv = np.asarray(wv, dtype=np.float32).astype(bf16)
    wo = np.asarray(wo, dtype=np.float32).astype(bf16)
    cos = np.asarray(cos, dtype=np.float32)
    sin = np.asarray(sin, dtype=np.float32)
    mask = np.asarray(mask, dtype=np.float32)
    p = int(pos)

    def pblock(a, nchunks):
        # [(chunks*128), n] -> [128, chunks, n] -> [128, chunks*n]
        n = a.shape[1]
        return np.ascontiguousarray(
            a.reshape(nchunks, P, n).transpose(1, 0, 2).reshape(P, nchunks * n)
        )

    cs = cos[p : p + S]  # [S, 32]
    sn = sin[p : p + S]
    cs9 = pblock(np.tile(cs, (1, NH + 1)), SC).astype(bf16)  # [128, 8*288]
    sn9 = pblock(np.tile(sn, (1, NH + 1)), SC).astype(bf16)
    # transposed diagonal 128x128 blocks of the mask as 0/1, tiled x8 heads
    mk8 = np.concatenate(
        [
            np.tile(
                (mask[s * P : (s + 1) * P, s * P : (s + 1) * P].T >= -0.5).astype(
                    bf16
                ),
                (1, NH),
            )
            for s in range(SC)
        ],
        axis=1,
    )
    mk8 = np.ascontiguousarray(mk8)  # [128, 8*1024]

    in_maps = []
    for d in range(N_CORES):
        g, kv = d // 4, d % 4
        in_maps.append(
            {
                "xt": np.ascontiguousarray(
                    x[g].T.reshape(DC, P, SC, P).transpose(2, 1, 0, 3).reshape(SC, P, D)
                ),
                "wq": pblock(wq[:, kv * 512 : (kv + 1) * 512], DC),
                "wkv": pblock(
                    np.concatenate(
                        [
                            wk[:, kv * HD : (kv + 1) * HD],
                            wv[:, kv * HD : (kv + 1) * HD],
                        ],
                        axis=1,
                    ),
                    DC,
                ),
                "wo": pblock(wo[:, kv * 512 : (kv + 1) * 512], DC),
                "cs9": cs9,
                "sn9": sn9,
                "mk8": mk8,
            }
        )
    return in_maps


def _run(inputs, trace=False, trace_kwargs=None):
    nc = _get_nc()
    in_maps = _shard_inputs(**inputs)
    res = run_bass_kernel_spmd(
        nc,
        in_maps,
        core_ids=list(range(N_CORES)),
        trace=trace,
        **(trace_kwargs or {}),
    )
    B = 2
    out = np.empty((B, S, D), dtype=np.float32)
    for d in range(N_CORES):
        g, kv = d // 4, d % 4
        out[g, :, kv * 512 : (kv + 1) * 512] = res.results[d]["out"]
    return out, res


def kernel(**inputs) -> np.ndarray:
    out, _ = _run(inputs, trace=False)
    return out
